# revision 1
# baseline (speedup 1.0000x reference)
"""Trainium2 Bass kernel for nn_Attention_6468220748045.

Computes, per batch item: QKV projection -> per-head scaled attention with a
multiplicative positional bias w[i,j] = |i-j|/S -> softmax -> attn @ V ->
LayerNorm over the embedding dim.

Sharding: pure data-parallel over batch. B=128 splits as 16 batch items per
core across 8 NeuronCores; no collectives needed. Inputs are pre-laid-out on
host: x is passed transposed per batch ([B, E, S]) so both projection
orientations stream directly from SBUF, and the weights are passed transposed
([e_in, e_out]) to serve as matmul stationary operands.

Schedule (from HW trace analysis): the attention matmuls (scores N=179,
PV N=65) are LDWEIGHTS-bound (~95ns per stationary load vs 27-75ns streams)
while projection matmuls are stream-bound (149-213ns) with weight loads fully
hidden.  So projections of pair p+1 interleave 1:1 at single-matmul
granularity into the attention of pair p, hiding attention LDWEIGHTS under
projection streams.  Within a batch, PV lags scores by two heads so the
softmax chain (VectorE w-mul -> ScalarE exp) always completes well before
the PE reaches the PV matmuls (the PE queue is strictly in-order; a stalled
head-of-line matmul starves everything, including the HAM clock gate).

Engine placement: all PSUM->SBUF evacuations (q/k/v projections) run on
ScalarE via activation-Copy; ScalarE otherwise only runs Exp, and Copy lives
in every ACT table, so the table loads exactly once (v1 paid 20 x 1.3us
swapping Exp<->Sqrt).  LayerNorm rstd uses a bit-trick + Newton rsqrt on
VectorE (no table function).  GpSimd handles the small memsets.

Known remaining headroom (designed, unlanded -- see below; plus two blocked
items): (a) ~50us: attention LDWEIGHTS serialize with their matmuls because
this toolchain hardcodes --enable-ldw-opt=false (no background weight-buffer
overlap); (b) ~13us: the projection-less final pair runs at HAM half-clock --
experimentally unfixable by scheduling (chain-bound slots cannot sustain the
clock gate's array-busy threshold).  (c) ~27us, IMPLEMENTABLE: offset-
partition V-tail packing.  The two s=128:179 V-projection stationary groups
per pair (51 useful rows each) can merge into ONE group via a strided
stationary [b0-tail 51 | 13 pad | b1-tail 51] IF batch-1's jt=1 data lives at
partitions 64:115 end-to-end: V-psum rows then land at 0:51 / 64:115 so both
evacuations are partition-ALIGNED (the usual partition-shift blocker
vanishes); scores jt=1 for batch 1 writes out=ps[64:115] (legal: output base
partition 64 is col-group aligned for <=64-row outputs); PV jt=1 for batch 1
slices both operands at base partition 64 (bases match, tile_position=(64,0)
legal); requires a second wsc plane with jt=1 rows at 64:115 and padding the
xt tile free dim by 13 cols so the strided stationary's pad-read cannot
overrun the tile at k=KT-1 (race-detector clean).  Cuts V-proj from 64 to 48
N=512 matmuls per pair.
"""

import numpy as np

import concourse.bass as bass
import concourse.tile as tile
from concourse import bacc, mybir
from concourse.bass_utils import run_bass_kernel_spmd

# Problem constants (hardcoded per the self-contained-kernel contract).
B, S, E, H, D = 128, 179, 1024, 16, 64
NCORES = 8
BPC = B // NCORES          # batches per core = 16
NPAIR = BPC // 2           # batch pairs per core = 8
KT = E // 128              # contraction tiles over e_in = 8
MT = E // 128              # output tiles over e_out = 8
S0 = 128                   # first s-tile size
S1 = S - S0                # second s-tile size = 51
S_TILES = ((0, S0), (S0, S1))
LN_EPS = 1e-5
SCALE = float(E) ** -0.5
PV_LAG = 2                 # heads by which PV trails scores

F32 = mybir.dt.float32
BF16 = mybir.dt.bfloat16
U32 = mybir.dt.uint32

AF = mybir.ActivationFunctionType
ALU = mybir.AluOpType


def _build_kernel(bpc: int = BPC, apply_gb: bool = True) -> bass.Bass:
    npair = bpc // 2
    nc = bacc.Bacc()

    xT = nc.dram_tensor("xT", [bpc, E, S], BF16, kind="ExternalInput").ap()
    wqT = nc.dram_tensor("wqT", [E, E], BF16, kind="ExternalInput").ap()
    wkT = nc.dram_tensor("wkT", [E, E], BF16, kind="ExternalInput").ap()
    wvT = nc.dram_tensor("wvT", [E, E], BF16, kind="ExternalInput").ap()
    wsc = nc.dram_tensor("wsc", [128, 2, S], F32, kind="ExternalInput").ap()
    gamma = nc.dram_tensor("gamma", [E], F32, kind="ExternalInput").ap()
    beta = nc.dram_tensor("beta", [E], F32, kind="ExternalInput").ap()
    out = nc.dram_tensor("out", [bpc, S, E], F32, kind="ExternalOutput").ap()

    with tile.TileContext(nc) as tc:
        _emit(tc, npair, out, xT, wqT, wkT, wvT, wsc, gamma, beta, apply_gb)
    nc.compile()
    return nc


def _emit(tc, npair, out, xT, wqT, wkT, wvT, wsc, gamma, beta, apply_gb):
    nc = tc.nc
    from contextlib import ExitStack

    with ExitStack() as ctx:
        singles = ctx.enter_context(tc.tile_pool(name="singles", bufs=1))
        xt_pool = ctx.enter_context(tc.tile_pool(name="xt", bufs=3))
        qk_pool = ctx.enter_context(tc.tile_pool(name="qk", bufs=3))
        v_pool = ctx.enter_context(tc.tile_pool(name="v", bufs=6))
        p_pool = ctx.enter_context(tc.tile_pool(name="p", bufs=6))
        o_pool = ctx.enter_context(tc.tile_pool(name="o", bufs=3))
        ln_pool = ctx.enter_context(tc.tile_pool(name="ln", bufs=4))
        r_pool = ctx.enter_context(tc.tile_pool(name="r", bufs=8))

        # PSUM (8 banks, every tile pads to one bank):
        # proj 3 + scores 3 + PV 2 = 8.
        pp_proj = ctx.enter_context(tc.tile_pool(name="pp_proj", bufs=3, space="PSUM"))
        pp_s = ctx.enter_context(tc.tile_pool(name="pp_s", bufs=3, space="PSUM"))
        pp_o = ctx.enter_context(tc.tile_pool(name="pp_o", bufs=2, space="PSUM"))

        # --- resident tensors -------------------------------------------------
        # Weight tiles: [e_in partition, k-tile, e_out]. DMA order matters for
        # startup latency: wq first, then pair-0's x.T, then wk/wv.
        xsrc = xT.rearrange("b (k p) s -> k p b s", p=128)  # [KT, 128, bpc, S]
        w_sbs = []
        for name in ("wq", "wk", "wv"):
            w_sb = singles.tile([128, KT, E], BF16, tag=f"w_{name}", name=f"w_{name}")
            w_sbs.append(w_sb)
        wq_sb, wk_sb, wv_sb = w_sbs
        xt0 = xt_pool.tile([128, KT, 2, S], BF16, tag="xt", name="xt_0")
        src = wqT.rearrange("(k p) e -> k p e", p=128)
        for k in range(KT):
            nc.sync.dma_start(out=wq_sb[:, k], in_=src[k])
            nc.sync.dma_start(out=xt0[:, k], in_=xsrc[k, :, 0:2, :])
        for w_sb, wap in ((wk_sb, wkT), (wv_sb, wvT)):
            src = wap.rearrange("(k p) e -> k p e", p=128)
            for k in range(KT):
                nc.sync.dma_start(out=w_sb[:, k], in_=src[k])

        # Positional bias (already includes softmax scale), host-precomputed as
        # [j mod 128, jt, i] with zero rows for j >= S.
        wsc_sb = singles.tile([128, 2, S], F32, tag="wsc")
        nc.sync.dma_start(out=wsc_sb, in_=wsc)

        if apply_gb:
            gamma_b = singles.tile([128, E], F32, tag="gamma")
            beta_b = singles.tile([128, E], F32, tag="beta")
            nc.sync.dma_start(
                out=gamma_b,
                in_=bass.AP(tensor=gamma.tensor, offset=gamma.offset, ap=[[0, 128]] + gamma.ap),
            )
            nc.sync.dma_start(
                out=beta_b,
                in_=bass.AP(tensor=beta.tensor, offset=beta.offset, ap=[[0, 128]] + beta.ap),
            )
        # Magic constant for the bit-trick rsqrt seed (no ACT table needed).
        magic_t = singles.tile([128, 1], U32, tag="magic")
        nc.vector.memset(magic_t, 0x5F3759DF)

        # Per-pair SBUF products handed from the projection stage to the
        # attention stage (software pipeline).  stage_qk lands after the QK
        # phase; stage_v[(pr, bi)] after that batch's V chunks, so the last
        # pair's V work can interleave into its own attention phase.
        stage_qk: dict = {}
        stage_v: dict = {}

        def proj_gen(pr):
            """QKV projections for batch pair `pr`; yields after each PE
            psum-group (~8 matmuls) so attention of pair pr-1 interleaves at
            chunk granularity (the PE weight-load path is one-deep, so finer
            interleave only serializes LDWEIGHTS of the two streams)."""
            if pr == 0:
                xt = xt0
            else:
                xt = xt_pool.tile([128, KT, 2, S], BF16, tag="xt", name=f"xt_{pr}")
                for k in range(KT):
                    nc.sync.dma_start(
                        out=xt[:, k], in_=xsrc[k, :, 2 * pr : 2 * pr + 2, :]
                    )

            # Q.T / K.T: out[e_out, s2], s2 = 2*S = 358 (both batches at once).
            qt_sb = qk_pool.tile([128, MT, 2, S], BF16, tag="qt", name=f"qt_{pr}")
            kt_sb = qk_pool.tile([128, MT, 2, S], BF16, tag="kt", name=f"kt_{pr}")
            for w_sb, dst in ((wq_sb, qt_sb), (wk_sb, kt_sb)):
                for m in range(MT):
                    ps = pp_proj.tile([128, 2, S], F32, tag="proj", name=f"psqk_{pr}_{m}")
                    for k in range(KT):
                        nc.tensor.matmul(
                            out=ps,
                            lhsT=w_sb[:, k, m * 128 : (m + 1) * 128],
                            rhs=xt[:, k],
                            start=(k == 0),
                            stop=(k == KT - 1),
                        )
                    # PSUM->SBUF evacuation on ScalarE (Copy: in every ACT
                    # table, so it never causes a table reload).
                    nc.scalar.copy(out=dst[:, m], in_=ps)
                    yield
            stage_qk[pr] = (qt_sb, kt_sb)

            # V: natural [s, e] layout with a ones column appended per head.
            vpads_by_b = [[None, None], [None, None]]
            for bi in range(2):
                for st, (ss, sn) in enumerate(S_TILES):
                    vp = v_pool.tile(
                        [128, H, D + 1], BF16, tag=f"vpad{st}", name=f"vp{st}_{pr}_{bi}"
                    )
                    nc.gpsimd.memset(vp[:sn, :, D : D + 1], 1.0)
                    vpads_by_b[bi][st] = vp
                    for n in range(2):
                        ps = pp_proj.tile(
                            [128, 512], F32, tag="proj", name=f"psv_{pr}_{bi}_{st}_{n}"
                        )
                        for k in range(KT):
                            nc.tensor.matmul(
                                out=ps[:sn],
                                lhsT=xt[:, k, bi, ss : ss + sn],
                                rhs=wv_sb[:, k, n * 512 : (n + 1) * 512],
                                start=(k == 0),
                                stop=(k == KT - 1),
                            )
                        nc.scalar.copy(
                            out=vp[:sn, n * 8 : (n + 1) * 8, 0:D],
                            in_=ps[:sn].rearrange("p (h d) -> p h d", d=D),
                        )
                        yield
                stage_v[(pr, bi)] = vpads_by_b[bi]

        def attn_gen(pr):
            """Attention + LayerNorm for both batches of pair `pr` (batch-
            major); yields per PE matmul for the 1:1 projection interleave.
            PV trails scores by PV_LAG heads."""
            qt_sb, kt_sb = stage_qk.pop(pr)
            o_by_b = []
            for bi in range(2):
                b = 2 * pr + bi
                o_by_b.append([
                    o_pool.tile([128, E], F32, tag=f"o{st}", name=f"o{st}_{b}")
                    for st, _ in enumerate(S_TILES)
                ])

            for bi in range(2):
                b = 2 * pr + bi
                yield ("need_v", pr, bi)
                vpads = stage_v.pop((pr, bi))
                o_tiles = o_by_b[bi]
                p_ts = {}
                ps_o4 = [None, None]

                def emit_scores(h):
                    m, r0 = h // 2, (h % 2) * D
                    ps_s = pp_s.tile([128, 2, S], F32, tag="s", name=f"pss_{b}_{h}")
                    nc.tensor.matmul(
                        out=ps_s[:, 0],
                        lhsT=kt_sb[r0 : r0 + D, m, bi, 0:128],
                        rhs=qt_sb[r0 : r0 + D, m, bi, :],
                        start=True,
                        stop=True,
                    )
                    nc.tensor.matmul(
                        out=ps_s[0:S1, 1],
                        lhsT=kt_sb[r0 : r0 + D, m, bi, 128:S],
                        rhs=qt_sb[r0 : r0 + D, m, bi, :],
                        start=True,
                        stop=True,
                    )
                    # Multiplicative bias + exp. Stale rows j>=S of the jt=1
                    # half see wsc=0 -> p=1; excluded by the :jn PV slices.
                    nc.vector.tensor_mul(out=ps_s, in0=ps_s, in1=wsc_sb)
                    p_t = p_pool.tile([128, 2, S], BF16, tag="p", name=f"p_{b}_{h}")
                    nc.scalar.activation(out=p_t, in_=ps_s, func=AF.Exp)
                    p_ts[h] = p_t

                def emit_pv(h):
                    hc = h % 4
                    p_t = p_ts.pop(h)
                    # PV: 4 heads share a psum bank: [i, 4, 65] where col 64
                    # of each head is the softmax denominator (ones col in V).
                    if hc == 0:
                        ps_o4[0] = pp_o.tile([128, 4, D + 1], F32, tag="po", name=f"pso_{b}_{h}_0")
                        ps_o4[1] = pp_o.tile([128, 4, D + 1], F32, tag="po", name=f"pso_{b}_{h}_1")
                    for it, (is_, in_n) in enumerate(S_TILES):
                        for jt, (js, jn) in enumerate(S_TILES):
                            nc.tensor.matmul(
                                out=ps_o4[it][:in_n, hc],
                                lhsT=p_t[:jn, jt, is_ : is_ + in_n],
                                rhs=vpads[jt][:jn, h],
                                start=(jt == 0),
                                stop=(jt == 1),
                            )
                    if hc == 3:
                        # Batched normalize for the 4-head group: one
                        # reciprocal of the 4 denominators, one broadcast
                        # multiply writing [i, 4*64] of the output tile.
                        g0 = (h - 3) * D
                        for it, (is_, in_n) in enumerate(S_TILES):
                            rec = r_pool.tile([128, 4], F32, tag="rec4", name=f"rc_{b}_{h}_{it}")
                            nc.vector.reciprocal(
                                out=rec[:in_n], in_=ps_o4[it][:in_n, :, D]
                            )
                            rb = rec[:in_n]
                            rbc = bass.AP(
                                tensor=rb.tensor,
                                offset=rb.offset,
                                ap=list(rb.ap) + [[0, D]],
                            )
                            nc.vector.tensor_mul(
                                out=o_tiles[it][:in_n, g0 : g0 + 4 * D].rearrange(
                                    "p (h d) -> p h d", d=D
                                ),
                                in0=ps_o4[it][:in_n, :, 0:D],
                                in1=rbc,
                            )

                for h in range(H):
                    emit_scores(h)
                    if h >= PV_LAG:
                        emit_pv(h - PV_LAG)
                    yield "h"
                for h in range(H - PV_LAG, H):
                    emit_pv(h)

            # LayerNorm for both batches last (VectorE stats + apply; rstd
            # via bit-trick + Newton rsqrt -- no ACT table function), kept out
            # of the inter-batch vector FIFO so it never delays the softmax
            # chain of the next batch.
            for bi in range(2):
                b = 2 * pr + bi
                for it, (is_, in_n) in enumerate(S_TILES):
                    o_sb = o_by_b[bi][it]
                    stats = ln_pool.tile([128, 2, 6], F32, tag="stats", name=f"st_{b}_{it}")
                    mv = ln_pool.tile([128, 2], F32, tag="mv", name=f"mv_{b}_{it}")
                    nc.vector.bn_stats(out=stats[:in_n, 0], in_=o_sb[:in_n, 0:512])
                    nc.vector.bn_stats(out=stats[:in_n, 1], in_=o_sb[:in_n, 512:E])
                    nc.vector.bn_aggr(out=mv[:in_n], in_=stats[:in_n])
                    ve = ln_pool.tile([128, 1], F32, tag="ve", name=f"ve_{b}_{it}")
                    nc.vector.tensor_scalar_add(ve[:in_n], mv[:in_n, 1:2], LN_EPS)
                    rstd = r_pool.tile([128, 1], F32, tag="rstd", name=f"rs_{b}_{it}")
                    nc.vector.tensor_scalar(
                        out=rstd[:in_n].bitcast(U32),
                        in0=ve[:in_n].bitcast(U32),
                        scalar1=1,
                        scalar2=None,
                        op0=ALU.logical_shift_right,
                    )
                    nc.vector.tensor_tensor(
                        out=rstd[:in_n].bitcast(U32),
                        in0=magic_t[:in_n],
                        in1=rstd[:in_n].bitcast(U32),
                        op=ALU.subtract,
                    )
                    t0 = r_pool.tile([128, 1], F32, tag="nt0", name=f"nt0_{b}_{it}")
                    for _ in range(2):
                        nc.vector.tensor_mul(out=t0[:in_n], in0=rstd[:in_n], in1=rstd[:in_n])
                        nc.vector.tensor_mul(out=t0[:in_n], in0=t0[:in_n], in1=ve[:in_n])
                        nc.vector.tensor_scalar(
                            out=t0[:in_n], in0=t0[:in_n],
                            scalar1=-0.5, scalar2=1.5, op0=ALU.mult, op1=ALU.add,
                        )
                        nc.vector.tensor_mul(out=rstd[:in_n], in0=rstd[:in_n], in1=t0[:in_n])
                    nc.vector.tensor_scalar(
                        out=o_sb[:in_n],
                        in0=o_sb[:in_n],
                        scalar1=mv[:in_n, 0:1],
                        scalar2=rstd[:in_n],
                        op0=ALU.subtract,
                        op1=ALU.mult,
                    )
                    if apply_gb:
                        nc.vector.tensor_mul(out=o_sb[:in_n], in0=o_sb[:in_n], in1=gamma_b[:in_n])
                        nc.vector.tensor_add(out=o_sb[:in_n], in0=o_sb[:in_n], in1=beta_b[:in_n])
                    nc.sync.dma_start(out=out[b, is_ : is_ + in_n], in_=o_sb[:in_n])
                yield "ln"

        # Software pipeline: attention(p) interleaved with projection chunks
        # of pair p+1.  The LAST pair's batch-1 V chunks are deferred into its
        # own attention phase so the tail keeps PE filler work (otherwise the
        # final attention runs bare and HAM re-throttles the clock).
        from collections import deque

        for _ in proj_gen(0):
            pass
        pending: deque = deque()
        next_pair = 1

        def push_next():
            nonlocal next_pair
            if next_pair < npair:
                pending.append((next_pair, proj_gen(next_pair)))
                next_pair += 1

        def advance_one(defer_tail=False):
            while pending:
                pr0, gen = pending[0]
                if (
                    defer_tail
                    and pr0 == npair - 1
                    and pr0 in stage_qk
                    and (pr0, 0) in stage_v
                ):
                    return False
                if next(gen, "END") == "END":
                    pending.popleft()
                    push_next()
                    continue
                return True
            return False

        push_next()
        for p in range(npair):
            ag = attn_gen(p)
            acc = 0
            defer = p < npair - 1
            for tok in ag:
                # Distribute proj chunks: 18 across the 32 head yields, 3 at
                # each LN yield (where attention gives the PE the least work).
                if isinstance(tok, tuple):
                    _, rp, rbi = tok
                    while (rp, rbi) not in stage_v:
                        if not advance_one():
                            break
                elif tok == "ln":
                    for _ in range(3):
                        if not advance_one(defer):
                            break
                else:
                    acc += 18
                    while acc >= 32:
                        if not advance_one(defer):
                            break
                        acc -= 32
            # Boundary: proj(p+1) QK must be emitted before attention(p+1).
            while (p + 1) < npair and (p + 1) not in stage_qk:
                if not advance_one():
                    break


_NC_CACHE: dict = {}


def _get_nc(bpc: int = BPC, apply_gb: bool = True) -> bass.Bass:
    key = (bpc, apply_gb)
    if key not in _NC_CACHE:
        _NC_CACHE[key] = _build_kernel(bpc, apply_gb)
    return _NC_CACHE[key]


def _host_inputs(x, Wq, Wk, Wv, gamma, beta):
    import ml_dtypes

    bf16 = ml_dtypes.bfloat16
    x = np.asarray(x, dtype=np.float32)
    xT = np.ascontiguousarray(x.transpose(0, 2, 1)).astype(bf16)  # [B, E, S]
    idx = np.arange(S, dtype=np.float32)
    w_full = (np.abs(idx[None, :] - idx[:, None]) / S * SCALE).astype(np.float32)
    # [j mod 128, jt, i] layout with zero rows for j >= S.
    wsc = np.zeros((128, 2, S), dtype=np.float32)
    wsc[0:128, 0] = w_full[0:128]
    wsc[0:S1, 1] = w_full[128:S]
    common = {
        "wqT": np.ascontiguousarray(np.asarray(Wq, np.float32).T).astype(bf16),
        "wkT": np.ascontiguousarray(np.asarray(Wk, np.float32).T).astype(bf16),
        "wvT": np.ascontiguousarray(np.asarray(Wv, np.float32).T).astype(bf16),
        "wsc": wsc,
        "gamma": np.asarray(gamma, np.float32),
        "beta": np.asarray(beta, np.float32),
    }
    return xT, common


def run(inputs: dict, trace: bool = False, trace_dir: str | None = None):
    """Run the SPMD kernel on 8 cores. Returns (full_output, exec_time_ns)."""
    xT, common = _host_inputs(**inputs)
    in_maps = [
        {**common, "xT": np.ascontiguousarray(xT[c * BPC : (c + 1) * BPC])}
        for c in range(NCORES)
    ]
    apply_gb = not (
        np.all(np.asarray(inputs["gamma"]) == 1.0)
        and np.all(np.asarray(inputs["beta"]) == 0.0)
    )
    nc = _get_nc(BPC, apply_gb)
    res = run_bass_kernel_spmd(
        nc, in_maps, core_ids=list(range(NCORES)), trace=trace, tmpdir=trace_dir
    )
    full = np.concatenate([res.results[c]["out"] for c in range(NCORES)], axis=0)
    return full.astype(np.float32), res.exec_time_ns


def kernel(x, Wq, Wk, Wv, gamma, beta):
    full, _ = run(dict(x=x, Wq=Wq, Wk=Wk, Wv=Wv, gamma=gamma, beta=beta))
    return full

